# revision 24
# baseline (speedup 1.0000x reference)
"""Trainium2 Bass kernel for 2D attention with relative-position augmentation.

Problem shapes (hardcoded): inputs [8, 32, 32, 768] fp32 (q|k|v packed on the
channel axis, 256 each), key_rel_w/key_rel_h [63, 32] fp32.
Output: [8, 32, 32, 256] fp32.

Sharding: data-parallel over batch - core b gets batch b (8 cores, no
collectives needed).

Per-core math (N = 32*32 = 1024 tokens, 8 heads, head dim 32):
  L[n, m] = Q[n].K[m] + qdw[n, y2(m)-y(n)+31] + qdh[n, x2(m)-x(n)+31]
  out[n]  = softmax_m(L[n, :] / sqrt(32)) @ V
where qdw = Q @ key_rel_w^T, qdh = Q @ key_rel_h^T and n=(x,y), m=(x2,y2).

v3 formulation:
  * L^T (m on partitions, n free) via ONE K=96 matmul per (head, m-chunk):
       lhsT rows  0-31: K^T          rhs rows  0-31: Q^T
       lhsT rows 32-63: Aw[y',m]     rhs rows 32-63: Bw[y',n]=qdw^T[y'-y(n)+31,n]
       lhsT rows 64-95: Ah[x',m]     rhs rows 64-95: Bh[x',n]
  * Aw/Ah one-hots, rel tables and PE-transpose identities are precomputed in
    numpy and passed as extra inputs.
  * Q^T/K^T: plain f32 HWDGE loads + fp32 PE transposes + cast evictions
    (ACT for Q, DVE for K) + SBUF->SBUF stitch DMAs into per-head blocks.
    (SWDGE cast DMAs execute ~3.4us each serially - kept off critical path;
    only the V staging uses them.)
  * B phase: stationary = shifted rel-table slice per (v, table), moving =
    all 8 heads' Q^T (N=256); 64 matmuls into [64,(v8,h8,u32)] PSUM groups.
  * exp split: ACT (exact Exp) on half the m-chunks, DVE Schraudolph fast-exp
    (bits16 = L*s + b as int16, bitcast to bf16) on the other half; the
    uniform half-step bias cancels in the softmax ratio.
  * AV: A^T = [V|1]^T P^T with V stationary (16 matmuls/head, N=512), evicted
    bf16 into [66, 512] (c-halves stacked on partitions), PE-transposed back
    two n-chunks at a time, then normalized: out = A[:, 0:32] * (1/A[:, 32]).
"""

import numpy as np

import concourse.bacc as bacc
import concourse.mybir as mybir
from concourse.tile import TileContext
from concourse.bass_utils import run_bass_kernel_spmd

F32 = mybir.dt.float32
BF16 = mybir.dt.bfloat16
I16 = mybir.dt.int16
AF = mybir.ActivationFunctionType
ALU = mybir.AluOpType

N_CORES = 8
N = 1024          # tokens per batch (32 x 32)
NH = 8            # heads
EXP_SCALE = float(1.0 / np.sqrt(32.0))
# Schraudolph constants for bf16 bit patterns: bits16 = L*SEXP_S + SEXP_B
SEXP_S = float(EXP_SCALE * 128.0 / np.log(2.0))
SEXP_B = float(127.0 * 128.0 - 7.42)
# which m-chunks use exact ACT exp (rest use DVE fast-exp)
ACT_CHUNKS = (0, 3, 4, 7)

_CACHE = {}


def _emit(tc, x, rtc, ohc, idf, idtb, out):
    nc = tc.nc

    with tc.tile_pool(name="big", bufs=1) as big:
        # ---- consts (scalar ring)
        rt = big.tile([32, 128], BF16, name="rt")
        idt = big.tile([33, 33], BF16, name="idt")
        idn = big.tile([128, 128], F32, name="idn")
        idnb = big.tile([128, 128], BF16, name="idnb")
        nc.scalar.dma_start(out=rt[:], in_=rtc[:])
        nc.scalar.dma_start(out=idt[:], in_=idtb[:])
        nc.scalar.dma_start(out=idn[:], in_=idf[:])
        nc.gpsimd.tensor_copy(idnb[:], idn[:])

        # ---- extended operand tiles; per-head 1024-col blocks.
        qe = big.tile([96, NH * N], BF16, name="qe")
        ke = big.tile([96, NH * N], BF16, name="ke")

        # ---- full-row contiguous x load (3KB runs; strided column loads
        # measured ~73GB/s), t-halves split across both HWDGE rings.
        x_r = x.rearrange("(t p) c -> p t c", p=128)
        vp = big.tile([128, 8 * NH * 33], BF16, name="vp")
        vp_r = vp[:].rearrange("p (t h c) -> p t h c", t=8, h=NH)
        with tc.tile_pool(name="xap", bufs=1) as xap, \
             tc.tile_pool(name="wpp", bufs=1, space="PSUM") as wpp, \
             tc.tile_pool(name="tpp", bufs=2, space="PSUM") as tpp:
            # HAM warmup: the PE is otherwise idle until the loads land
            # (~14us), guaranteeing a cold 1.2GHz start whose recovery time
            # is phase-luck (measured 28-63us!). Hammer self-contained junk
            # matmuls so the clock-gate opens by ~11us on every run.
            warm = xap.tile([128, 128], BF16, name="warm")
            nc.gpsimd.memset(warm[:], 1.0)
            wps = wpp.tile([128, 512], F32, name="wps")
            for w in range(104):
                nc.tensor.matmul(
                    wps[:, (w % 4) * 128:(w % 4 + 1) * 128],
                    warm[:], warm[:], start=True, stop=True,
                )
            xall = xap.tile([128, 8 * 768], F32, name="xall")
            xa_r = xall[:].rearrange("p (t c) -> p t c", c=768)
            for tq in range(4):
                eng = nc.sync if tq < 2 else nc.scalar
                ts = slice(tq * 2, tq * 2 + 2)
                eng.dma_start(out=xa_r[:, ts, :], in_=x_r[:, ts, :])
            # pre-cast Q|K columns to bf16 so the PE transposes run at
            # 1 cycle/row instead of fp32's 2
            xb = xap.tile([128, 8 * 512], BF16, name="xb")
            xb_r = xb[:].rearrange("p (t c) -> p t c", c=512)
            nc.vector.tensor_copy(xb_r[:, 0:4, :], xa_r[:, 0:4, 0:512])
            nc.scalar.activation(xb_r[:, 4:8, :], xa_r[:, 4:8, 0:512], AF.Copy)
            # bf16 PE transposes -> strips [128 (4 heads), 1024] bf16;
            # Q first so its stitches overlap the K transposes.
            qt0 = big.tile([128, N], BF16, name="qt0")
            qt1 = big.tile([128, N], BF16, name="qt1")
            kt0 = big.tile([128, N], BF16, name="kt0")
            kt1 = big.tile([128, N], BF16, name="kt1")
            for c0, st0, st1, evict, dst_e in (
                (0, qt0, qt1, "act", qe), (256, kt0, kt1, "dve", ke),
            ):
                for cb in range(2):
                    st = st0 if cb == 0 else st1
                    for th in range(2):
                        tp = tpp.tile([128, 512], BF16, name="tp")
                        for t in range(4):
                            nc.tensor.transpose(
                                tp[:, t * 128:(t + 1) * 128],
                                xb_r[:, th * 4 + t,
                                     c0 + cb * 128:c0 + (cb + 1) * 128],
                                idnb[:],
                            )
                        dst = st[:, th * 512:(th + 1) * 512]
                        if evict == "act":
                            nc.scalar.activation(dst, tp[:], AF.Copy)
                        else:
                            nc.vector.tensor_copy(dst, tp[:])
                # stitch strips into per-head blocks, ring-alternated
                for h in range(NH):
                    stt = st0 if h < 4 else st1
                    p0 = (h % 4) * 32
                    eng = nc.sync if h % 2 == 0 else nc.scalar
                    eng.dma_start(
                        out=dst_e[0:32, h * N:(h + 1) * N],
                        in_=stt[p0:p0 + 32, :],
                    )
            # one-hot rows of ke (after the stitch triggers so those fire
            # as early as possible; oh has no dependencies)
            for h in range(NH):
                eng = nc.sync if h % 2 == 0 else nc.scalar
                eng.dma_start(out=ke[32:96, h * N:(h + 1) * N], in_=ohc[:])
            # V staging on ACT (keeps DVE free for K evicts / B evicts;
            # SWDGE cast DMAs run ~3.4us each, serially)
            xv4 = xa_r.rearrange("p t (h c) -> p t h c", h=24)
            nc.scalar.activation(vp_r[:, :, :, 0:32], xv4[:, :, 16:24, :], AF.Copy)
            nc.vector.memset(vp_r[:, :, :, 32:33], 1.0)
            # PE warmup filler sized ~to the stitch latency: keep HAM
            # un-throttled into the B phase
            for w in range(4):
                tp = tpp.tile([128, 512], BF16, name="tp")
                for t in range(4):
                    nc.tensor.transpose(
                        tp[:, t * 128:(t + 1) * 128], idnb[:], idnb[:],
                    )

        # ---- B phase: Bw/Bh rows of qe via shifted free-slices of rt.
        #   Bw[y', n]|y(n)=v = rt[:, 31-v:63-v].T @ Q^T[:, n]  (u = x free)
        #   Bh[x', n]|x(n)=v = rt[:, 95-v:127-v].T @ Q^T[:, n] (u = y free)
        # One matmul per (v, table) over ALL heads (N=256); PSUM groups of
        # 8 v-values: [64, (v8, h8, u32)].
        qe_v = qe[:].rearrange("p (h nx ny) -> p h nx ny", h=NH, nx=32)
        with tc.tile_pool(name="bpp", bufs=4, space="PSUM") as bpp:
            for g in range(8):
                b_ps = bpp.tile([64, 1024], F32, name="b_ps")
                for vi in range(4):
                    v = g * 4 + vi
                    nc.tensor.matmul(
                        b_ps[0:32, vi * 256:(vi + 1) * 256],
                        rt[:, 31 - v:63 - v],
                        qe_v[0:32, :, :, v:v + 1],
                        start=True, stop=True,
                    )
                    nc.tensor.matmul(
                        b_ps[32:64, vi * 256:(vi + 1) * 256],
                        rt[:, 95 - v:127 - v],
                        qe_v[0:32, :, v:v + 1, :],
                        start=True, stop=True,
                    )
                bw = b_ps[0:32, :].rearrange("p (v h u) -> p h u v", v=4, h=NH)
                bh = b_ps[32:64, :].rearrange("p (v h u) -> p h v u", v=4, h=NH)
                vs = slice(g * 4, g * 4 + 4)
                nc.scalar.activation(qe_v[32:64, :, :, vs], bw, AF.Copy)
                nc.vector.tensor_copy(qe_v[64:96, :, vs, :], bh)

        # ---- main loop
        out_r = out.rearrange("(j p) c -> p j c", p=128)
        with tc.tile_pool(name="lpp", bufs=2, space="PSUM") as lpp, \
             tc.tile_pool(name="atp", bufs=2, space="PSUM") as atp, \
             tc.tile_pool(name="trp", bufs=2, space="PSUM") as trp, \
             tc.tile_pool(name="ptp", bufs=6) as ptp, \
             tc.tile_pool(name="asb", bufs=2) as asb, \
             tc.tile_pool(name="outp", bufs=4) as outp:
            def emit_tail_a(at_sb):
                # first half of the transpose-back; split across two emission
                # points so the PE filler lands in both exp-wait windows
                tr_ps = trp.tile([128, 8 * 36], BF16, name="tr_ps")
                for j in range(4):
                    nc.tensor.transpose(
                        tr_ps[:, j * 36:j * 36 + 33],
                        at_sb[:, j * 128:(j + 1) * 128],
                        idt[:],
                    )
                return tr_ps

            def emit_tail(h, at_sb, tr_ps):
                for j in range(4, 8):
                    nc.tensor.transpose(
                        tr_ps[:, j * 36:j * 36 + 33],
                        at_sb[:, j * 128:(j + 1) * 128],
                        idt[:],
                    )
                tr_r = tr_ps[:].rearrange("p (j c) -> p j c", c=36)
                r = outp.tile([128, 8], F32, name="r")
                r_r = r[:].rearrange("p (j o) -> p j o", o=1)
                nc.vector.reciprocal(r_r, tr_r[:, :, 32:33])
                o_sb = outp.tile([128, 256], F32, name="o_sb")
                o_r = o_sb[:].rearrange("p (j c) -> p j c", c=32)
                nc.vector.tensor_tensor(
                    o_r, tr_r[:, :, 0:32], r_r.broadcast_to([128, 8, 32]),
                    ALU.mult,
                )
                nc.sync.dma_start(
                    out=out_r[:, :, h * 32:(h + 1) * 32], in_=o_r[:, :, :]
                )

            pend = None
            for h in range(NH):
                pts = []
                for i in range(8):
                    l_ps = lpp.tile([128, N], F32, name="l_ps")
                    for c in range(2):
                        nc.tensor.matmul(
                            l_ps[:, c * 512:(c + 1) * 512],
                            ke[:, h * N + i * 128: h * N + i * 128 + 128],
                            qe[:, h * N + c * 512: h * N + (c + 1) * 512],
                            start=True, stop=True,
                        )
                    if i in ACT_CHUNKS:
                        pt = ptp.tile([128, N], BF16, name="pt")
                        nc.scalar.activation(pt[:], l_ps[:], AF.Exp, scale=EXP_SCALE)
                        pts.append(pt[:])
                    else:
                        pt = ptp.tile([128, N], I16, name="pti")
                        nc.vector.tensor_scalar(
                            pt[:], l_ps[:], SEXP_S, SEXP_B, ALU.mult, ALU.add
                        )
                        pts.append(pt[:].bitcast(BF16))
                    if i == 1 and pend is not None:
                        tr_pend = emit_tail_a(pend)
                    if i == 3 and pend is not None:
                        emit_tail(h - 1, pend, tr_pend)
                        pend = None
                # A^T = [V|1]^T @ P^T accumulated over the 8 m-chunks, in
                # two 512-col halves (one PSUM bank each).
                at_sb = asb.tile([33, N], BF16, name="at_sb")
                for c in range(2):
                    at_ps = atp.tile([33, 512], F32, name="at_ps")
                    for i in range(8):
                        nc.tensor.matmul(
                            at_ps[:],
                            vp_r[:, i, h, :],
                            pts[i][:, c * 512:(c + 1) * 512],
                            start=(i == 0), stop=(i == 7),
                        )
                    dst = at_sb[:, c * 512:(c + 1) * 512]
                    if c == 0:
                        nc.scalar.activation(dst, at_ps[:], AF.Copy)
                    else:
                        nc.vector.tensor_copy(dst, at_ps[:])
                pend = at_sb
            emit_tail(NH - 1, pend, emit_tail_a(pend))


def build_nc():
    if "nc" in _CACHE:
        return _CACHE["nc"]
    nc = bacc.Bacc(
        "TRN2", target_bir_lowering=False, debug=False, num_devices=N_CORES
    )
    x = nc.dram_tensor("x", [N, 768], F32, kind="ExternalInput")
    rtc = nc.dram_tensor("rtc", [32, 128], BF16, kind="ExternalInput")
    ohc = nc.dram_tensor("ohc", [64, N], BF16, kind="ExternalInput")
    idf = nc.dram_tensor("idf", [128, 128], F32, kind="ExternalInput")
    idtb = nc.dram_tensor("idtb", [33, 33], BF16, kind="ExternalInput")
    out = nc.dram_tensor("out", [N, 256], F32, kind="ExternalOutput")
    with TileContext(nc) as tc:
        _emit(tc, x.ap(), rtc.ap(), ohc.ap(), idf.ap(), idtb.ap(), out.ap())
    nc.compile()
    _CACHE["nc"] = nc
    return nc


def make_in_maps(inputs, key_rel_w, key_rel_h):
    bf = mybir.dt.np(BF16)
    B = inputs.shape[0]
    x_full = np.ascontiguousarray(inputs.reshape(B, N, 768), dtype=np.float32)
    rtc = np.zeros((32, 128), np.float32)
    rtc[:, 0:63] = np.asarray(key_rel_w, np.float32).T
    rtc[:, 64:127] = np.asarray(key_rel_h, np.float32).T
    rtc = np.ascontiguousarray(rtc.astype(bf))
    m = np.arange(N)
    ohc = np.zeros((64, N), np.float32)
    ohc[0:32] = (m[None, :] % 32) == np.arange(32)[:, None]
    ohc[32:64] = (m[None, :] // 32) == np.arange(32)[:, None]
    ohc = np.ascontiguousarray(ohc.astype(bf))
    idf = np.ascontiguousarray(np.eye(128, dtype=np.float32))
    idtb = np.ascontiguousarray(np.eye(33, dtype=np.float32).astype(bf))
    return [
        {"x": x_full[b], "rtc": rtc, "ohc": ohc, "idf": idf, "idtb": idtb}
        for b in range(B)
    ]


def kernel(inputs, key_rel_w, key_rel_h):
    assert inputs.shape == (8, 32, 32, 768), inputs.shape
    nc = build_nc()
    in_maps = make_in_maps(inputs, key_rel_w, key_rel_h)
    res = run_bass_kernel_spmd(nc, in_maps, list(range(N_CORES)))
    return np.stack(
        [res.results[b]["out"].reshape(32, 32, 256) for b in range(N_CORES)]
    )


if __name__ == "__main__":
    rng = np.random.default_rng(0)
    inputs = rng.standard_normal((8, 32, 32, 768), dtype=np.float32)
    rw = rng.standard_normal((63, 32), dtype=np.float32) * 32 ** -0.5
    rh = rng.standard_normal((63, 32), dtype=np.float32) * 32 ** -0.5
    o = kernel(inputs, rw, rh)
    print(o.shape, o.dtype, float(np.abs(o).max()))


# revision 26
# speedup vs baseline: 1.2121x; 1.2121x over previous
"""Trainium2 Bass kernel for 2D attention with relative-position augmentation.

Problem shapes (hardcoded): inputs [8, 32, 32, 768] fp32 (q|k|v packed on the
channel axis, 256 each), key_rel_w/key_rel_h [63, 32] fp32.
Output: [8, 32, 32, 256] fp32.

Sharding: data-parallel over batch - core b gets batch b (8 cores, no
collectives needed).

Per-core math (N = 32*32 = 1024 tokens, 8 heads, head dim 32):
  L[n, m] = Q[n].K[m] + qdw[n, y2(m)-y(n)+31] + qdh[n, x2(m)-x(n)+31]
  out[n]  = softmax_m(L[n, :] / sqrt(32)) @ V
where qdw = Q @ key_rel_w^T, qdh = Q @ key_rel_h^T and n=(x,y), m=(x2,y2).

v3 formulation:
  * L^T (m on partitions, n free) via ONE K=96 matmul per (head, m-chunk):
       lhsT rows  0-31: K^T          rhs rows  0-31: Q^T
       lhsT rows 32-63: Aw[y',m]     rhs rows 32-63: Bw[y',n]=qdw^T[y'-y(n)+31,n]
       lhsT rows 64-95: Ah[x',m]     rhs rows 64-95: Bh[x',n]
  * Aw/Ah one-hots, rel tables and PE-transpose identities are precomputed in
    numpy and passed as extra inputs.
  * Q^T/K^T: plain f32 HWDGE loads + fp32 PE transposes + cast evictions
    (ACT for Q, DVE for K) + SBUF->SBUF stitch DMAs into per-head blocks.
    (SWDGE cast DMAs execute ~3.4us each serially - kept off critical path;
    only the V staging uses them.)
  * B phase: stationary = shifted rel-table slice per (v, table), moving =
    all 8 heads' Q^T (N=256); 64 matmuls into [64,(v8,h8,u32)] PSUM groups.
  * exp split: ACT (exact Exp) on half the m-chunks, DVE Schraudolph fast-exp
    (bits16 = L*s + b as int16, bitcast to bf16) on the other half; the
    uniform half-step bias cancels in the softmax ratio.
  * AV: A^T = [V|1]^T P^T with V stationary (16 matmuls/head, N=512), evicted
    bf16 into [66, 512] (c-halves stacked on partitions), PE-transposed back
    two n-chunks at a time, then normalized: out = A[:, 0:32] * (1/A[:, 32]).
"""

import numpy as np

import concourse.bacc as bacc
import concourse.mybir as mybir
from concourse.tile import TileContext
from concourse.bass_utils import run_bass_kernel_spmd

F32 = mybir.dt.float32
BF16 = mybir.dt.bfloat16
I16 = mybir.dt.int16
AF = mybir.ActivationFunctionType
ALU = mybir.AluOpType

N_CORES = 8
N = 1024          # tokens per batch (32 x 32)
NH = 8            # heads
EXP_SCALE = float(1.0 / np.sqrt(32.0))
# Schraudolph constants for bf16 bit patterns: bits16 = L*SEXP_S + SEXP_B
SEXP_S = float(EXP_SCALE * 128.0 / np.log(2.0))
SEXP_B = float(127.0 * 128.0 - 7.42)
# which m-chunks use exact ACT exp (rest use DVE fast-exp)
ACT_CHUNKS = (0, 2, 4, 6)

_CACHE = {}


def _emit(tc, x, rtc, ohc, idf, idtb, out):
    nc = tc.nc

    with tc.tile_pool(name="big", bufs=1) as big:
        # ---- consts (scalar ring)
        rt = big.tile([32, 128], BF16, name="rt")
        idt = big.tile([33, 33], BF16, name="idt")
        idn = big.tile([128, 128], F32, name="idn")
        idnb = big.tile([128, 128], BF16, name="idnb")
        nc.scalar.dma_start(out=rt[:], in_=rtc[:])
        nc.scalar.dma_start(out=idt[:], in_=idtb[:])
        nc.scalar.dma_start(out=idn[:], in_=idf[:])
        nc.gpsimd.tensor_copy(idnb[:], idn[:])

        # ---- extended operand tiles; per-head 1024-col blocks.
        qe = big.tile([96, NH * N], BF16, name="qe")
        ke = big.tile([96, NH * N], BF16, name="ke")

        # ---- full-row contiguous x load (3KB runs; strided column loads
        # measured ~73GB/s), t-halves split across both HWDGE rings.
        x_r = x.rearrange("(t p) c -> p t c", p=128)
        vp = big.tile([128, 8 * NH * 33], BF16, name="vp")
        vp_r = vp[:].rearrange("p (t h c) -> p t h c", t=8, h=NH)
        with tc.tile_pool(name="xap", bufs=1) as xap, \
             tc.tile_pool(name="wpp", bufs=1, space="PSUM") as wpp, \
             tc.tile_pool(name="tpp", bufs=2, space="PSUM") as tpp:
            # HAM warmup: the PE is otherwise idle until the loads land
            # (~14us), guaranteeing a cold 1.2GHz start whose recovery time
            # is phase-luck (measured 28-63us!). Hammer self-contained junk
            # matmuls so the clock-gate opens by ~11us on every run.
            warm = xap.tile([128, 128], BF16, name="warm")
            nc.gpsimd.memset(warm[:], 1.0)
            wps = wpp.tile([128, 512], F32, name="wps")
            for w in range(104):
                nc.tensor.matmul(
                    wps[:, (w % 4) * 128:(w % 4 + 1) * 128],
                    warm[:], warm[:], start=True, stop=True,
                )
            xall = xap.tile([128, 8 * 768], F32, name="xall")
            xa_r = xall[:].rearrange("p (t c) -> p t c", c=768)
            for tq in range(4):
                eng = nc.sync if tq < 2 else nc.scalar
                ts = slice(tq * 2, tq * 2 + 2)
                eng.dma_start(out=xa_r[:, ts, :], in_=x_r[:, ts, :])
            # pre-cast Q|K columns to bf16 so the PE transposes run at
            # 1 cycle/row instead of fp32's 2
            xb = xap.tile([128, 8 * 512], BF16, name="xb")
            xb_r = xb[:].rearrange("p (t c) -> p t c", c=512)
            nc.vector.tensor_copy(xb_r[:, 0:4, :], xa_r[:, 0:4, 0:512])
            nc.scalar.activation(xb_r[:, 4:8, :], xa_r[:, 4:8, 0:512], AF.Copy)
            # bf16 PE transposes -> strips [128 (4 heads), 1024] bf16;
            # Q first so its stitches overlap the K transposes.
            qt0 = big.tile([128, N], BF16, name="qt0")
            qt1 = big.tile([128, N], BF16, name="qt1")
            kt0 = big.tile([128, N], BF16, name="kt0")
            kt1 = big.tile([128, N], BF16, name="kt1")
            for c0, st0, st1, evict, dst_e in (
                (0, qt0, qt1, "act", qe), (256, kt0, kt1, "dve", ke),
            ):
                for cb in range(2):
                    st = st0 if cb == 0 else st1
                    for th in range(2):
                        tp = tpp.tile([128, 512], BF16, name="tp")
                        for t in range(4):
                            nc.tensor.transpose(
                                tp[:, t * 128:(t + 1) * 128],
                                xb_r[:, th * 4 + t,
                                     c0 + cb * 128:c0 + (cb + 1) * 128],
                                idnb[:],
                            )
                        dst = st[:, th * 512:(th + 1) * 512]
                        if evict == "act":
                            nc.scalar.activation(dst, tp[:], AF.Copy)
                        else:
                            nc.vector.tensor_copy(dst, tp[:])
                # stitch strips into per-head blocks, ring-alternated
                for h in range(NH):
                    stt = st0 if h < 4 else st1
                    p0 = (h % 4) * 32
                    eng = nc.sync if h % 2 == 0 else nc.scalar
                    eng.dma_start(
                        out=dst_e[0:32, h * N:(h + 1) * N],
                        in_=stt[p0:p0 + 32, :],
                    )
            # one-hot rows of ke (after the stitch triggers so those fire
            # as early as possible; oh has no dependencies)
            for h in range(NH):
                eng = nc.sync if h % 2 == 0 else nc.scalar
                eng.dma_start(out=ke[32:96, h * N:(h + 1) * N], in_=ohc[:])
            # V staging on ACT (keeps DVE free for K evicts / B evicts;
            # SWDGE cast DMAs run ~3.4us each, serially)
            xv4 = xa_r.rearrange("p t (h c) -> p t h c", h=24)
            nc.scalar.activation(vp_r[:, :, :, 0:32], xv4[:, :, 16:24, :], AF.Copy)
            nc.vector.memset(vp_r[:, :, :, 32:33], 1.0)
            # PE warmup filler sized ~to the stitch latency: keep HAM
            # un-throttled into the B phase
            for w in range(4):
                tp = tpp.tile([128, 512], BF16, name="tp")
                for t in range(4):
                    nc.tensor.transpose(
                        tp[:, t * 128:(t + 1) * 128], idnb[:], idnb[:],
                    )

        # ---- B phase: Bw/Bh rows of qe via shifted free-slices of rt.
        #   Bw[y', n]|y(n)=v = rt[:, 31-v:63-v].T @ Q^T[:, n]  (u = x free)
        #   Bh[x', n]|x(n)=v = rt[:, 95-v:127-v].T @ Q^T[:, n] (u = y free)
        # One matmul per (v, table) over ALL heads (N=256); PSUM groups of
        # 8 v-values: [64, (v8, h8, u32)].
        qe_v = qe[:].rearrange("p (h nx ny) -> p h nx ny", h=NH, nx=32)
        with tc.tile_pool(name="bpp", bufs=4, space="PSUM") as bpp:
            for g in range(8):
                b_ps = bpp.tile([64, 1024], F32, name="b_ps")
                for vi in range(4):
                    v = g * 4 + vi
                    nc.tensor.matmul(
                        b_ps[0:32, vi * 256:(vi + 1) * 256],
                        rt[:, 31 - v:63 - v],
                        qe_v[0:32, :, :, v:v + 1],
                        start=True, stop=True,
                    )
                    nc.tensor.matmul(
                        b_ps[32:64, vi * 256:(vi + 1) * 256],
                        rt[:, 95 - v:127 - v],
                        qe_v[0:32, :, v:v + 1, :],
                        start=True, stop=True,
                    )
                bw = b_ps[0:32, :].rearrange("p (v h u) -> p h u v", v=4, h=NH)
                bh = b_ps[32:64, :].rearrange("p (v h u) -> p h v u", v=4, h=NH)
                vs = slice(g * 4, g * 4 + 4)
                nc.scalar.activation(qe_v[32:64, :, :, vs], bw, AF.Copy)
                nc.vector.tensor_copy(qe_v[64:96, :, vs, :], bh)

        # ---- main loop
        out_r = out.rearrange("(j p) c -> p j c", p=128)
        with tc.tile_pool(name="lpp", bufs=2, space="PSUM") as lpp, \
             tc.tile_pool(name="atp", bufs=3, space="PSUM") as atp, \
             tc.tile_pool(name="trp", bufs=1, space="PSUM") as trp, \
             tc.tile_pool(name="ptp", bufs=10) as ptp, \
             tc.tile_pool(name="asb", bufs=2) as asb, \
             tc.tile_pool(name="outp", bufs=4) as outp:
            def emit_tail(h, at_sb):
                # transpose back to [n, 33] per n-chunk, then normalize.
                # Emitted AFTER the next head's first logits so the PE never
                # stalls on the at_sb evictions.
                tr_ps = trp.tile([128, 8 * 36], BF16, name="tr_ps")
                for j in range(8):
                    nc.tensor.transpose(
                        tr_ps[:, j * 36:j * 36 + 33],
                        at_sb[:, j * 128:(j + 1) * 128],
                        idt[:],
                    )
                tr_r = tr_ps[:].rearrange("p (j c) -> p j c", c=36)
                r = outp.tile([128, 8], F32, name="r")
                r_r = r[:].rearrange("p (j o) -> p j o", o=1)
                nc.vector.reciprocal(r_r, tr_r[:, :, 32:33])
                o_sb = outp.tile([128, 256], F32, name="o_sb")
                o_r = o_sb[:].rearrange("p (j c) -> p j c", c=32)
                nc.vector.tensor_tensor(
                    o_r, tr_r[:, :, 0:32], r_r.broadcast_to([128, 8, 32]),
                    ALU.mult,
                )
                nc.sync.dma_start(
                    out=out_r[:, :, h * 32:(h + 1) * 32], in_=o_r[:, :, :]
                )

            pend = None
            for h in range(NH):
                pts = []
                for i in range(8):
                    l_ps = lpp.tile([128, N], F32, name="l_ps")
                    for c in range(2):
                        nc.tensor.matmul(
                            l_ps[:, c * 512:(c + 1) * 512],
                            ke[:, h * N + i * 128: h * N + i * 128 + 128],
                            qe[:, h * N + c * 512: h * N + (c + 1) * 512],
                            start=True, stop=True,
                        )
                    if i in ACT_CHUNKS:
                        pt = ptp.tile([128, N], BF16, name="pt")
                        nc.scalar.activation(pt[:], l_ps[:], AF.Exp, scale=EXP_SCALE)
                        pts.append(pt[:])
                    else:
                        pt = ptp.tile([128, N], I16, name="pti")
                        nc.vector.tensor_scalar(
                            pt[:], l_ps[:], SEXP_S, SEXP_B, ALU.mult, ALU.add
                        )
                        pts.append(pt[:].bitcast(BF16))
                    if i == 1 and pend is not None:
                        emit_tail(h - 1, pend)
                        pend = None
                # A^T = [V|1]^T @ P^T accumulated over the 8 m-chunks, in
                # two 512-col halves (one PSUM bank each).
                at_sb = asb.tile([33, N], BF16, name="at_sb")
                for c in range(2):
                    at_ps = atp.tile([33, 512], F32, name="at_ps")
                    for i in range(8):
                        nc.tensor.matmul(
                            at_ps[:],
                            vp_r[:, i, h, :],
                            pts[i][:, c * 512:(c + 1) * 512],
                            start=(i == 0), stop=(i == 7),
                        )
                    dst = at_sb[:, c * 512:(c + 1) * 512]
                    if c == 0:
                        nc.scalar.activation(dst, at_ps[:], AF.Copy)
                    else:
                        nc.vector.tensor_copy(dst, at_ps[:])
                pend = at_sb
            emit_tail(NH - 1, pend)


def build_nc():
    if "nc" in _CACHE:
        return _CACHE["nc"]
    nc = bacc.Bacc(
        "TRN2", target_bir_lowering=False, debug=False, num_devices=N_CORES
    )
    x = nc.dram_tensor("x", [N, 768], F32, kind="ExternalInput")
    rtc = nc.dram_tensor("rtc", [32, 128], BF16, kind="ExternalInput")
    ohc = nc.dram_tensor("ohc", [64, N], BF16, kind="ExternalInput")
    idf = nc.dram_tensor("idf", [128, 128], F32, kind="ExternalInput")
    idtb = nc.dram_tensor("idtb", [33, 33], BF16, kind="ExternalInput")
    out = nc.dram_tensor("out", [N, 256], F32, kind="ExternalOutput")
    with TileContext(nc) as tc:
        _emit(tc, x.ap(), rtc.ap(), ohc.ap(), idf.ap(), idtb.ap(), out.ap())
    nc.compile()
    _CACHE["nc"] = nc
    return nc


def make_in_maps(inputs, key_rel_w, key_rel_h):
    bf = mybir.dt.np(BF16)
    B = inputs.shape[0]
    x_full = np.ascontiguousarray(inputs.reshape(B, N, 768), dtype=np.float32)
    rtc = np.zeros((32, 128), np.float32)
    rtc[:, 0:63] = np.asarray(key_rel_w, np.float32).T
    rtc[:, 64:127] = np.asarray(key_rel_h, np.float32).T
    rtc = np.ascontiguousarray(rtc.astype(bf))
    m = np.arange(N)
    ohc = np.zeros((64, N), np.float32)
    ohc[0:32] = (m[None, :] % 32) == np.arange(32)[:, None]
    ohc[32:64] = (m[None, :] // 32) == np.arange(32)[:, None]
    ohc = np.ascontiguousarray(ohc.astype(bf))
    idf = np.ascontiguousarray(np.eye(128, dtype=np.float32))
    idtb = np.ascontiguousarray(np.eye(33, dtype=np.float32).astype(bf))
    return [
        {"x": x_full[b], "rtc": rtc, "ohc": ohc, "idf": idf, "idtb": idtb}
        for b in range(B)
    ]


def kernel(inputs, key_rel_w, key_rel_h):
    assert inputs.shape == (8, 32, 32, 768), inputs.shape
    nc = build_nc()
    in_maps = make_in_maps(inputs, key_rel_w, key_rel_h)
    res = run_bass_kernel_spmd(nc, in_maps, list(range(N_CORES)))
    return np.stack(
        [res.results[b]["out"].reshape(32, 32, 256) for b in range(N_CORES)]
    )


if __name__ == "__main__":
    rng = np.random.default_rng(0)
    inputs = rng.standard_normal((8, 32, 32, 768), dtype=np.float32)
    rw = rng.standard_normal((63, 32), dtype=np.float32) * 32 ** -0.5
    rh = rng.standard_normal((63, 32), dtype=np.float32) * 32 ** -0.5
    o = kernel(inputs, rw, rh)
    print(o.shape, o.dtype, float(np.abs(o).max()))
